# revision 4
# baseline (speedup 1.0000x reference)
"""Trainium2 Bass kernel for nn_GATt_to_R_78950088835242 (GNN message passing).

Math: with rel_size = arange(E), x_res2[rel_size] is the identity, and the
per-relation softmax weights alpha sum to 1 within each segment, so
    x_type[rel] == x_res2 == M2[rel],
where M2 = concat(mean_h, mean_t) @ W_sr1 + b_sr1 and mean_h/mean_t are the
per-relation means of s_t[src]/s_t[dst].  Further, the t_c1 projection
commutes with the segment mean:  mean_h = mean(x_e[src]) @ W_tc1 + b_tc1.
So the output is
    out[e] = [ x_res1[e] + M2[r] | mean_h[r] | mean_t[r] ]   with r = rel[e],
all derived from raw-feature segment sums A_h/A_t and host-folded weights.

Sharding: edges are bucketed by rel // 125 so core c owns relations
[125c, 125c+125); per-relation tables are <= 128 rows (SBUF-resident), no
collectives.  Per core, edges are sorted by rel and padded so relation r
occupies exactly rows [640r, 640(r+1)) of the device edge arrays: the
edge->relation map becomes a compile-time constant, so pass 2 needs no
gather machinery at all.

Device pipeline per core (SPMD, no cross-core traffic):
  pass 1: stream the core's COMPACT node table (only nodes its edges touch)
          as fp8 + DoubleRow matmuls accumulating A = x_e^T @ [Mh|Mt].
  stage D: tiny matmuls fold A into M2^T (f16, [feat, rel]) and a fp8
          [rel, 256] mean table, which is partition-broadcast into a
          replicated copy for output DMAs.
  pass 2: out_a^T[:, seg_r] = x_res1^T[:, seg_r] + M2^T[:, r] via DVE/ACT
          per-partition-bias adds (f16); out_b rows are written by plain
          DMAs replicating each relation's 256-byte row over its segment.
"""

import math
import os
import sys
import time
import types

import numpy as np


def _ensure_ntff_hook():
    """This image's antenv lacks axon_hooks; inject a shim and register the
    ctypes NTFF profile hook so trace=True can report HW exec time."""
    if "antenv.axon_hooks" in sys.modules:
        return
    mod = types.ModuleType("antenv.axon_hooks")
    mod._hook = None

    def set_axon_ntff_profile_hook(h):
        mod._hook = h

    def get_axon_ntff_profile_hook():
        return mod._hook

    mod.set_axon_ntff_profile_hook = set_axon_ntff_profile_hook
    mod.get_axon_ntff_profile_hook = get_axon_ntff_profile_hook
    sys.modules["antenv.axon_hooks"] = mod
    try:
        from trn_agent_boot.trn_boot import _ntff_profile_via_ctypes

        hook = _ntff_profile_via_ctypes("/opt/axon/libaxon_pjrt.so")
        if hook is not None:
            mod._hook = hook
    except Exception:
        pass


_ensure_ntff_hook()

N_NODES = 100000
E_TOTAL = 500000
NUM_REL = 1000
E_HID = 256
T_HID = 128
R_HID = 128
N_CORES = 8
RPC = NUM_REL // N_CORES  # 125 relations per core
P = 128
NB = 28  # node tiles per pass-1 DMA batch
CAP = 640  # per-relation edge-segment capacity (5 * 128)
E_PAD = RPC * CAP  # 80000
COLB = 5 * CAP  # pass-2 column batch = 5 segments = 3200 edges
NBATCH = E_PAD // COLB  # 25

OUT_W = 3 * R_HID  # 384
OUT_A = R_HID  # f16 cols [0:128)
OUT_B = 2 * T_HID  # fp8 cols [128:384)


def _build_program(nu_pad: int, debug_outputs: bool = False):
    from concourse import bacc, mybir, tile

    f32 = mybir.dt.float32
    f16 = mybir.dt.float16
    f8 = mybir.dt.float8e4
    AOT = mybir.AluOpType
    DR = mybir.MatmulPerfMode.DoubleRow

    nc = bacc.Bacc(
        "TRN2", target_bir_lowering=False, debug=False, num_devices=N_CORES
    )

    xe8 = nc.dram_tensor("xe8", [nu_pad, E_HID], f8, kind="ExternalInput")
    mcat = nc.dram_tensor("mcat", [nu_pad, E_HID], f8, kind="ExternalInput")
    rho_in = nc.dram_tensor("rho", [P, 1], f32, kind="ExternalInput")
    xr1t = nc.dram_tensor("xr1t", [P, E_PAD], f16, kind="ExternalInput")
    vh = nc.dram_tensor("vh", [E_HID, R_HID], f16, kind="ExternalInput")
    vt = nc.dram_tensor("vt", [E_HID, R_HID], f16, kind="ExternalInput")
    w1 = nc.dram_tensor("w1", [E_HID, T_HID], f16, kind="ExternalInput")
    crep = nc.dram_tensor("crep", [P, OUT_W], f32, kind="ExternalInput")
    idm = nc.dram_tensor("idm", [P, P], f32, kind="ExternalInput")
    out_at = nc.dram_tensor("out_at", [P, E_PAD], f16, kind="ExternalOutput")
    out_b = nc.dram_tensor("out_b", [E_PAD, OUT_B], f8, kind="ExternalOutput")
    if debug_outputs:
        dbg_a = nc.dram_tensor("dbg_a", [P, 4 * P], f32, kind="ExternalOutput")
        dbg_m2t = nc.dram_tensor("dbg_m2t", [P, P], f32, kind="ExternalOutput")
        dbg_tb = nc.dram_tensor("dbg_tb", [P, OUT_B], f32, kind="ExternalOutput")

    with tile.TileContext(nc) as tc:
        with tc.tile_pool(name="const", bufs=1) as cp:
            rho_t = cp.tile([P, 1], f32, tag="rho")
            nc.sync.dma_start(out=rho_t[:], in_=rho_in[:])
            crep_t = cp.tile([P, OUT_W], f32, tag="crep")
            nc.sync.dma_start(out=crep_t[:], in_=crep[:])
            idm_t = cp.tile([P, P], f32, tag="idm")
            nc.sync.dma_start(out=idm_t[:], in_=idm[:])
            wts = {}
            for nm, h in (("vh", vh), ("vt", vt), ("w1", w1)):
                for k in range(2):
                    t_ = cp.tile([P, T_HID], f16, tag=f"{nm}{k}")
                    nc.sync.dma_start(out=t_[:], in_=h[k * P : (k + 1) * P, :])
                    wts[f"{nm}{k}"] = t_
            m2t = cp.tile([P, P], f32, tag="m2t")  # [feat, rel], stage D
            tb8 = cp.tile([P, OUT_B], f8, tag="tb8")  # [rel, 256], stage D
            trep = cp.tile([P, RPC, 3, OUT_B], f8, tag="trep")  # bcast copies

            with tc.tile_pool(name="psA", bufs=1, space="PSUM") as psA:
                A = psA.tile([P, 4 * P], f32, tag="A")
                n_nsuper = nu_pad // (NB * P)

                # ---- pass 1: A = x_e^T @ [Mh | Mt] over compact node tiles.
                # p-major rearrange: partition p reads NB contiguous rows.
                with tc.tile_pool(name="p1x", bufs=3) as p1x, \
                     tc.tile_pool(name="p1m", bufs=3) as p1m:
                    for ns in range(n_nsuper):
                        base = ns * NB * P
                        xt = p1x.tile([P, NB, E_HID], f8, tag="xt")
                        nc.sync.dma_start(
                            out=xt[:],
                            in_=xe8[base : base + NB * P].rearrange(
                                "(p j) f -> p j f", j=NB
                            ),
                        )
                        mt = p1m.tile([P, NB, E_HID], f8, tag="mt")
                        nc.sync.dma_start(
                            out=mt[:],
                            in_=mcat[base : base + NB * P].rearrange(
                                "(p j) f -> p j f", j=NB
                            ),
                        )
                        for j in range(0, NB, 2):
                            for k in range(2):
                                first = ns == 0 and j == 0 and k == 0
                                last = (
                                    ns == n_nsuper - 1 and j == NB - 2 and k == 1
                                )
                                nc.tensor.matmul(
                                    out=A[:, k * 2 * P : (k + 1) * 2 * P],
                                    lhsT=xt[:, j : j + 2, k * P : (k + 1) * P],
                                    rhs=mt[:, j : j + 2, :],
                                    start=first,
                                    stop=last,
                                    perf_mode=DR,
                                    skip_group_check=True,
                                )

                # ---------------- stage D: build the tables ----------------
                with tc.tile_pool(name="sd", bufs=1) as sd, \
                     tc.tile_pool(name="psD", bufs=1, space="PSUM") as psD:
                    # A layout: [Ah0 | At0 | Ah1 | At1] (feat chunk f0/f1 rows)
                    atiles = []
                    for k in range(4):
                        a_ = sd.tile([P, P], f16, tag=f"A{k}")
                        nc.vector.tensor_copy(out=a_[:], in_=A[:, k * P : (k + 1) * P])
                        atiles.append(a_)
                    ah0, at0, ah1, at1 = atiles
                    S = psD.tile([P, OUT_W], f32, tag="S")
                    blocks = {
                        0: [(ah0, "vh0"), (ah1, "vh1"), (at0, "vt0"), (at1, "vt1")],
                        1: [(ah0, "w10"), (ah1, "w11")],
                        2: [(at0, "w10"), (at1, "w11")],
                    }
                    for b, lst in blocks.items():
                        for i, (a, w) in enumerate(lst):
                            nc.tensor.matmul(
                                out=S[:, b * P : (b + 1) * P],
                                lhsT=a[:],
                                rhs=wts[w][:],
                                start=(b == 0 and i == 0),
                                stop=(b == 2 and i == len(lst) - 1),
                                skip_group_check=True,
                            )
                    ssc = sd.tile([P, OUT_W], f32, tag="ssc")
                    nc.vector.tensor_scalar_mul(ssc[:], S[:], rho_t[:])
                    # table rows with biases folded in
                    ta32 = sd.tile([P, OUT_A], f32, tag="ta32")
                    nc.vector.tensor_tensor(
                        out=ta32[:], in0=ssc[:, :OUT_A], in1=crep_t[:, :OUT_A],
                        op=AOT.add,
                    )
                    nc.vector.tensor_tensor(
                        out=tb8[:], in0=ssc[:, OUT_A:], in1=crep_t[:, OUT_A:],
                        op=AOT.add,
                    )
                    # M2^T via PE transpose
                    pT = psD.tile([P, P], f32, tag="pT")
                    nc.tensor.transpose(out=pT[:], in_=ta32[:], identity=idm_t[:])
                    nc.vector.tensor_copy(out=m2t[:], in_=pT[:])
                    # replicate tb8 across partitions (3 interleaved copies)
                    g1 = sd.tile([1, RPC, OUT_B], f8, tag="g1")
                    nc.sync.dma_start(out=g1[:], in_=tb8[0:RPC, :])
                    for s in range(3):
                        nc.gpsimd.partition_broadcast(
                            out_ap=trep[:, :, s, :], in_ap=g1[:], channels=P
                        )
                    if debug_outputs:
                        da = sd.tile([P, 4 * P], f32, tag="dbg_a_s")
                        nc.vector.tensor_copy(out=da[:], in_=A[:])
                        nc.sync.dma_start(out=dbg_a[:], in_=da[:])
                        dm = sd.tile([P, P], f32, tag="dbg_m2t_s")
                        nc.vector.tensor_copy(out=dm[:], in_=pT[:])
                        nc.sync.dma_start(out=dbg_m2t[:], in_=dm[:])
                        db = sd.tile([P, OUT_B], f32, tag="dbg_tb_s")
                        nc.vector.tensor_copy(out=db[:], in_=tb8[:])
                        nc.sync.dma_start(out=dbg_tb[:], in_=db[:])

            # ---------------- pass 2: emit output ----------------
            # out_a^T columns: per 640-col segment, add M2^T[:, r] as a
            # per-partition scalar.  out_b rows: replication DMAs from trep.
            with tc.tile_pool(name="p2x", bufs=3) as p2x, \
                 tc.tile_pool(name="p2o", bufs=3) as p2o:
                for b in range(NBATCH):
                    c0 = b * COLB
                    xrt = p2x.tile([P, COLB], f16, tag="xrt")
                    nc.sync.dma_start(out=xrt[:], in_=xr1t[:, c0 : c0 + COLB])
                    oat = p2o.tile([P, COLB], f16, tag="oat")
                    for k in range(5):
                        r = 5 * b + k
                        sl = slice(k * CAP, (k + 1) * CAP)
                        if k < 3:
                            nc.vector.tensor_scalar_add(
                                oat[:, sl], xrt[:, sl], m2t[:, r : r + 1]
                            )
                        else:
                            nc.scalar.add(
                                oat[:, sl], xrt[:, sl], add=m2t[:, r : r + 1]
                            )
                        # out_b segment r: 640 rows = 384 + 256, p-major
                        ob0 = r * CAP
                        nc.sync.dma_start(
                            out=out_b[ob0 : ob0 + 384].rearrange(
                                "(p c) f -> p c f", c=3
                            ),
                            in_=trep[:, r, :, :],
                        )
                        nc.scalar.dma_start(
                            out=out_b[ob0 + 384 : ob0 + CAP].rearrange(
                                "(p c) f -> p c f", c=2
                            ),
                            in_=trep[:, r, 0:2, :],
                        )
                    nc.scalar.dma_start(
                        out=out_at[:, c0 : c0 + COLB], in_=oat[:]
                    )

    nc.compile()
    return nc


def _host_prep(x_e, x_res1, W_tc1, b_tc1, W_sr1, b_sr1, edge_index, rel):
    """Bucket edges by relation, sort into fixed-capacity segments, build
    per-core compact node tables and input maps (index-only + dtype prep)."""
    x_e = np.asarray(x_e, dtype=np.float32)
    x_res1 = np.asarray(x_res1, dtype=np.float32)
    W_tc1 = np.asarray(W_tc1, dtype=np.float32)
    b_tc1 = np.asarray(b_tc1, dtype=np.float32)
    W_sr1 = np.asarray(W_sr1, dtype=np.float32)
    b_sr1 = np.asarray(b_sr1, dtype=np.float32)
    edge_index = np.asarray(edge_index)
    rel = np.asarray(rel)

    shard_of = rel // RPC
    idx_per_core = [np.flatnonzero(shard_of == c) for c in range(N_CORES)]

    # Host-folded weight products (constant folding of the two Linears).
    vh = (W_tc1 @ W_sr1[:T_HID]).astype(np.float16)  # [256, 128]
    vt = (W_tc1 @ W_sr1[T_HID:]).astype(np.float16)  # [256, 128]
    w1 = W_tc1.astype(np.float16)  # [256, 128]
    b_eff = b_tc1 @ (W_sr1[:T_HID] + W_sr1[T_HID:]) + b_sr1  # [128]
    const_row = np.concatenate([b_eff, b_tc1, b_tc1]).astype(np.float32)  # [384]
    crep = np.broadcast_to(const_row, (P, OUT_W)).copy()

    import ml_dtypes

    f8 = ml_dtypes.float8_e4m3
    xe8_full = x_e.astype(f8)

    src = np.ascontiguousarray(edge_index[0]).astype(np.int64)
    dst = np.ascontiguousarray(edge_index[1]).astype(np.int64)

    per_core = []
    nu_max = 0
    for c in range(N_CORES):
        ix = idx_per_core[c]
        rel_loc = (rel[ix] - c * RPC).astype(np.int64)
        order = np.argsort(rel_loc, kind="stable")
        ixs = ix[order]
        rls = rel_loc[order]
        counts = np.bincount(rls, minlength=RPC)
        assert counts.max() <= CAP, f"segment overflow: {counts.max()} > {CAP}"
        cumstarts = np.concatenate([[0], np.cumsum(counts)[:-1]])
        within = np.arange(len(ixs)) - np.repeat(cumstarts, counts)
        pos = np.repeat(np.arange(RPC) * CAP, counts) + within
        nodes_c = np.unique(np.concatenate([src[ixs], dst[ixs]]))
        nu_max = max(nu_max, len(nodes_c))
        per_core.append((ixs, rls, counts, pos, nodes_c))

    nu_pad = math.ceil(nu_max / (NB * P)) * (NB * P)

    consts = dict(
        vh=vh, vt=vt, w1=w1, crep=crep, idm=np.eye(P, dtype=np.float32)
    )

    in_maps = []
    for c in range(N_CORES):
        ixs, rls, counts, pos, nodes_c = per_core[c]
        nu = len(nodes_c)

        xe8 = np.zeros((nu_pad, E_HID), dtype=f8)
        xe8[:nu] = xe8_full[nodes_c]

        isrc = np.searchsorted(nodes_c, src[ixs])
        idst = np.searchsorted(nodes_c, dst[ixs])

        # Incidence-count matrix on compact node ids.
        mint = np.zeros(nu_pad * E_HID, dtype=np.int32)
        np.add.at(mint, isrc * E_HID + rls, 1)
        np.add.at(mint, idst * E_HID + T_HID + rls, 1)
        assert mint.max() <= 16, "fp8 count overflow"
        mcat = mint.reshape(nu_pad, E_HID).astype(f8)

        cnt = np.zeros(P, dtype=np.float64)
        cnt[:RPC] = counts
        rho = (1.0 / np.maximum(cnt, 1.0)).astype(np.float32)[:, None]

        xr1t = np.zeros((P, E_PAD), dtype=np.float16)
        xr1t[:, pos] = x_res1[ixs].T

        m = dict(xe8=xe8, mcat=mcat, rho=rho, xr1t=xr1t, **consts)
        in_maps.append(m)
    return in_maps, per_core, nu_pad


_prog_cache: dict[int, object] = {}

last_exec_time_ns = None
last_results = None


def kernel(
    x_e,
    x_res1,
    W_tc1,
    b_tc1,
    W_sr1,
    b_sr1,
    a1,
    a5,
    edge_index,
    rel,
    rel_size,
):
    global last_exec_time_ns, last_results
    from concourse.bass_utils import run_bass_kernel_spmd

    in_maps, per_core, nu_pad = _host_prep(
        x_e, x_res1, W_tc1, b_tc1, W_sr1, b_sr1, edge_index, rel
    )

    if nu_pad not in _prog_cache:
        t0 = time.time()
        _prog_cache[nu_pad] = _build_program(nu_pad)
        print(f"[kernel] built+compiled program in {time.time() - t0:.1f}s")
    nc = _prog_cache[nu_pad]

    trace = os.environ.get("KBENCH_TRACE", "1") == "1"
    t0 = time.time()
    res = run_bass_kernel_spmd(nc, in_maps, list(range(N_CORES)), trace=trace)
    print(f"[kernel] device run (incl staging) {time.time() - t0:.1f}s")
    last_exec_time_ns = getattr(res, "exec_time_ns", None)
    last_results = res

    out = np.empty((E_TOTAL, OUT_W), dtype=np.float32)
    for c in range(N_CORES):
        ixs, rls, counts, pos, nodes_c = per_core[c]
        oat = res.results[c]["out_at"]  # [128, E_PAD] f16
        ob = res.results[c]["out_b"]  # [E_PAD, 256] fp8
        out[ixs, :OUT_A] = oat[:, pos].T.astype(np.float32)
        out[ixs, OUT_A:] = ob[pos].astype(np.float32)
    return out
